# revision 1
# baseline (speedup 1.0000x reference)
"""CenterLoss kernel for Trainium2 (8 NeuronCores, Bass/Tile).

Math: the reference builds the full [B, C] squared-distance matrix
    dist[b, c] = ||f_b||^2 + ||c_c||^2 - 2 f_b . c_c
masks it with (labels[b] == c), clamps to [1e-12, 1e12] and takes
sum/B.  The mask keeps exactly one entry per row (b, labels[b]); every
masked-out zero clamps to the constant 1e-12.  Hence

    loss = ( sum_b clip(||f_b - c_{l_b}||^2, 1e-12, 1e12)
             + (B*C - B) * 1e-12 ) / B

so only the B gathered distances need computing.  Sharding: batch is
split across the 8 cores (512 rows each); every core holds the full
centers table in HBM and gathers its 512 label rows with indirect DMA,
then computes row-wise squared distances, clamps, and reduces to a
per-core scalar partial.  The host sums the 8 partials (the scalar
all-reduce step) and applies the closed-form clamp constant.

Per-core on-chip layout: SBUF partition p holds batch rows 4p..4p+3 of
the core's shard, so the feature DMA moves one contiguous block per
partition in a single instruction and each of the 4 gather calls uses
one label column [128, 1].  Features and centers are streamed as bf16
(the gather phase is SDMA-descriptor-latency bound, and halving the
bytes shrinks the data tail); the distance accumulation stays fp32,
leaving ~1e-6 relative error on the final scalar.
"""

import numpy as np

B = 4096
C = 10000
D = 512
N_CORES = 8
ROWS_PER_CORE = B // N_CORES  # 512
P = 128
TILES = ROWS_PER_CORE // P  # 4
CLAMP_LO = 1e-12
CLAMP_HI = 1e12

_CACHED_NC = None


def _build_module():
    import concourse.bass as bass
    import concourse.mybir as mybir
    import concourse.tile as tile
    from concourse import bacc

    nc = bacc.Bacc(
        "TRN2",
        target_bir_lowering=False,
        debug=False,
        num_devices=N_CORES,
        dynamic_dma_scratch_size=2**16,
    )

    feats = nc.dram_tensor(
        "feats", [P, TILES * D], mybir.dt.bfloat16, kind="ExternalInput"
    ).ap()
    labs = nc.dram_tensor(
        "labs", [P, TILES], mybir.dt.int32, kind="ExternalInput"
    ).ap()
    ctrs = nc.dram_tensor(
        "centers", [C, D], mybir.dt.bfloat16, kind="ExternalInput"
    ).ap()
    out = nc.dram_tensor(
        "partial", [1, 1], mybir.dt.float32, kind="ExternalOutput"
    ).ap()

    with tile.TileContext(nc) as tc:
        with (
            tc.tile_pool(name="sb", bufs=1) as sb,
            tc.tile_pool(name="work", bufs=TILES) as work,
            tc.tile_pool(name="psum", bufs=1, space="PSUM") as psum,
        ):
            # Labels ride the otherwise-empty HWDGE ring: RTL descriptor
            # generation gives them a head start over the SWDGE-generated
            # feature stream, so their completion sem fires before any bulk
            # packets hit the SDMA engines.  Features + gathers share the
            # SWDGE FIFO (features first, which the gathers' desc-gen
            # overlaps).
            l_sb = sb.tile([P, TILES], mybir.dt.int32, tag="l")
            nc.sync.dma_start(out=l_sb[:], in_=labs[:])
            f_sb = sb.tile([P, TILES * D], mybir.dt.bfloat16, tag="f")
            nc.gpsimd.dma_start(out=f_sb[:], in_=feats[:])

            dists = sb.tile([P, TILES], mybir.dt.float32, tag="dist")
            for n in range(TILES):
                ct = work.tile([P, D], mybir.dt.bfloat16, tag="ct")
                nc.gpsimd.indirect_dma_start(
                    out=ct[:],
                    out_offset=None,
                    in_=ctrs[:],
                    in_offset=bass.IndirectOffsetOnAxis(
                        ap=l_sb[:, n : n + 1], axis=0
                    ),
                )
                df = work.tile([P, D], mybir.dt.float32, tag="df")
                nc.vector.tensor_tensor(
                    out=df[:],
                    in0=f_sb[:, n * D : (n + 1) * D],
                    in1=ct[:],
                    op=mybir.AluOpType.subtract,
                )
                sq = work.tile([P, D], mybir.dt.bfloat16, tag="sq")
                nc.scalar.activation(
                    out=sq[:],
                    in_=df[:],
                    func=mybir.ActivationFunctionType.Square,
                    accum_out=dists[:, n : n + 1],
                )

            dc = sb.tile([P, TILES], mybir.dt.float32, tag="dc")
            nc.vector.tensor_scalar(
                out=dc[:],
                in0=dists[:],
                scalar1=CLAMP_LO,
                scalar2=CLAMP_HI,
                op0=mybir.AluOpType.max,
                op1=mybir.AluOpType.min,
            )
            # partition-reduce via PE: ones[128,1].T @ dc[128,4] -> [1,4]
            # column sums in PSUM, then a tiny free-axis reduce straight to
            # SBUF — one DVE op shorter than reduce-then-matmul-then-copy
            ones = sb.tile([P, 1], mybir.dt.float32, tag="ones")
            nc.vector.memset(ones[:], 1.0)
            acc = psum.tile([1, TILES], mybir.dt.float32)
            nc.tensor.matmul(
                out=acc[:], lhsT=ones[:], rhs=dc[:], start=True, stop=True
            )
            res = sb.tile([1, 1], mybir.dt.float32, tag="res")
            nc.vector.reduce_sum(
                out=res[:], in_=acc[:], axis=mybir.AxisListType.X
            )
            nc.sync.dma_start(out=out[:], in_=res[:])

    nc.compile()
    return nc


def _get_module():
    global _CACHED_NC
    if _CACHED_NC is None:
        _CACHED_NC = _build_module()
    return _CACHED_NC


def _make_in_maps(features, labels, centers):
    import ml_dtypes

    bf16 = ml_dtypes.bfloat16
    f = np.ascontiguousarray(np.asarray(features)).astype(bf16)
    l = np.ascontiguousarray(np.asarray(labels)).astype(np.int32)
    c = np.ascontiguousarray(np.asarray(centers)).astype(bf16)
    f_sh = f.reshape(N_CORES, P, TILES * D)
    l_sh = l.reshape(N_CORES, P, TILES)
    return [
        {"feats": f_sh[k], "labs": l_sh[k], "centers": c} for k in range(N_CORES)
    ]


def run_spmd(features, labels, centers, **kwargs):
    """Compile (cached) + run on the 8 cores; returns BassKernelResults."""
    from concourse.bass_utils import run_bass_kernel_spmd

    nc = _get_module()
    in_maps = _make_in_maps(features, labels, centers)
    return run_bass_kernel_spmd(nc, in_maps, core_ids=list(range(N_CORES)), **kwargs)


def _combine(results):
    total = float(sum(float(r["partial"][0, 0]) for r in results))
    total += (B * C - B) * CLAMP_LO  # clamped masked-out zeros
    return np.array(total / B, dtype=np.float32)


def kernel(features, labels, centers):
    import time

    last = None
    for attempt in range(3):
        try:
            br = run_spmd(features, labels, centers)
            return _combine(br.results)
        except Exception as e:  # transient device wedge: back off and retry
            last = e
            time.sleep(2.0 * (attempt + 1))
    raise last

